# revision 8
# baseline (speedup 1.0000x reference)
"""Trainium2 Bass kernel for CompositionalTwoArmedAgent (DND-LSTM A2C step).

Strategy (8 NeuronCores, column-sharded DND, ZERO device collectives):
  - vals is sharded COLUMN-wise: core k owns all 100k rows x hidden cols
    [128k:128(k+1)], stored fp8e4 in a row-chunked DoubleRow layout. Each
    core computes its own slice of p = e @ vals completely locally.
  - keys are host-normalized (khat = k/||k||, fp8) and replicated; each
    core computes the full similarity vector e = exp(khat @ qhat) with a
    10-step DVE multiply-accumulate chain + fused exp/rowsum on ScalarE.
    Pad rows store khat = -40*qhat so their weight underflows to exactly 0.
  - Cosine sims are bounded in [-1,1] => no softmax max-pass needed.
  - The LSTM gates are sliced to match: core k computes preact rows
    {g*1024 + 128k + p} for the 5 gates via 45 small stationary-weight
    matmuls (contraction over the replicated h / x), so f,i,o,r,c_new,
    c_t, h_t for its slice need no cross-core data at all.
  - The big matvec p_slice = e @ vals_slice runs as 392 DoubleRow fp8
    matmuls (256 rows per step, 2 MACs/PE/cycle) accumulating in PSUM.
  - All loads ride ONE HWDGE ring (sync) in need-order so completion
    semaphores fire as soon as each tensor lands (two rings round-robin
    at packet granularity and starve each other's completions).
  - e is indexed g' = s*392 + c (s = DoubleRow subtile, c = chunk) so the
    ScalarE exp writes it contiguously while the per-chunk ldweights AP
    keeps the required 16-byte-multiple subtile stride.
  - Host gathers the 8 [h_t|c_t] slices and applies the tiny A2C head
    (relu GEMV + 2-class softmax + fixed-key categorical sample), as the
    baseline already did for the softmax/sample.
"""

import ml_dtypes
import numpy as np

import concourse.bacc as bacc
import concourse.bass as bass
import concourse.mybir as mybir
import concourse.tile as tile
from concourse.bass_utils import run_bass_kernel_spmd

N_CORES = 8
D, RD, H, IN_DIM, A = 100000, 10, 1024, 14, 2
PAD_D = 100352          # 392 * 256
NCH = 392               # DoubleRow chunks (256 rows each)
G10 = 784               # 128-row groups (= 2 * NCH)
NBLK = 7                # vals DMA blocks
CPB = NCH // NBLK       # chunks per block
F32 = mybir.dt.float32
BF16 = mybir.dt.bfloat16
FP8 = mybir.dt.float8e4
FP8_NP = ml_dtypes.float8_e4m3

# jax.random.gumbel(jax.random.key(1), (2,), float32) — fixed constants of the
# reference's categorical sample (verified against jax.random.categorical).
GUMBEL = np.array([0.5325072, -0.01641824], np.float32)

_CACHE = {}


def _input_specs():
    return [
        ("vals_s", [128, NCH * 2 * 128], FP8),   # DoubleRow-chunked vals shard
        ("khat", [128, RD * G10], FP8),          # normalized keys, r-major
        ("pk32", [128, 16], F32),                # q_bc(10) | b_sl(5) | c2_sl(1)
        ("pk16", [128, 9], BF16),                # h_col8(8) | x_col(col 8)
        ("wg", [128, 5 * 8 * 128], BF16),        # gate h2h weights [g][kc][p]
        ("wx", [14, 5 * 128], BF16),             # gate i2h weights [g][p]
    ]


def _build():
    nc = bacc.Bacc("TRN2", target_bir_lowering=False, debug=False,
                   num_devices=N_CORES)
    d = {name: nc.dram_tensor(name, shp, dt, kind="ExternalInput")
         for name, shp, dt in _input_specs()}
    out_hc = nc.dram_tensor("out_hc", [128, 2], F32, kind="ExternalOutput")

    AF = mybir.ActivationFunctionType
    OP = mybir.AluOpType
    DR = mybir.MatmulPerfMode.DoubleRow

    with tile.TileContext(nc) as tc:
        with (
            tc.tile_pool(name="const", bufs=1) as cp,
            tc.tile_pool(name="ps", bufs=1, space="PSUM") as pp,
        ):
            # ---- single sync-ring DMA queue, in need-order ------------
            khat_sb = cp.tile([128, RD, G10], FP8)
            nc.sync.dma_start(
                khat_sb[:], d["khat"][:].rearrange("p (r g) -> p r g", g=G10))
            pk32_sb = cp.tile([128, 16], F32)
            nc.sync.dma_start(pk32_sb[:], d["pk32"][:])
            pk16_sb = cp.tile([128, 9], BF16)
            nc.sync.dma_start(pk16_sb[:], d["pk16"][:])
            wg_sb = cp.tile([128, 5, 8, 128], BF16)
            nc.sync.dma_start(
                wg_sb[:], d["wg"][:].rearrange("p (g c j) -> p g c j",
                                               c=8, j=128))
            wx_sb = cp.tile([14, 5, 128], BF16)
            nc.sync.dma_start(
                wx_sb[:], d["wx"][:].rearrange("p (g j) -> p g j", j=128))
            vblk = []
            for b in range(NBLK):
                v = cp.tile([128, CPB, 2, 128], FP8)
                src = d["vals_s"][:, b * CPB * 256:(b + 1) * CPB * 256]
                nc.sync.dma_start(
                    v[:], src.rearrange("p (c s j) -> p c s j", s=2, j=128))
                vblk.append(v)

            q_ap = pk32_sb[:, 0:RD]
            b_ap = pk32_sb[:, RD:RD + 5]
            c2_ap = pk32_sb[:, RD + 5:RD + 6]
            h8_ap = pk16_sb[:, 0:8]
            x_ap = pk16_sb[0:14, 8:9]

            ones_sb = cp.tile([1, 1], F32)
            nc.vector.memset(ones_sb[:], 1.0)

            # ---- ACT table warmup (exp + tanh) ------------------------
            warm = cp.tile([1, 1], F32)
            nc.vector.memset(warm[:], 0.0)
            w2 = cp.tile([1, 1], F32)
            nc.scalar.activation(w2[:], warm[:], AF.Exp)
            nc.scalar.activation(w2[:], warm[:], AF.Tanh)

            # ---- e = exp(khat @ qhat), fused rowsum -------------------
            acc0 = cp.tile([128, G10], F32)
            acc1 = cp.tile([128, G10], F32)
            nc.vector.tensor_scalar(acc0[:], khat_sb[:, 0, :],
                                    q_ap[:, 0:1], None, OP.mult)
            accs = [acc0, acc1]
            for r in range(1, RD):
                nc.vector.scalar_tensor_tensor(
                    accs[r % 2][:], khat_sb[:, r, :], q_ap[:, r:r + 1],
                    accs[(r - 1) % 2][:], OP.mult, OP.add)
            # [128, 2, 400]: k-subtile-major so the DoubleRow ldweights AP
            # has a pair stride of 400 (multiple of 16, per s3_lw_dual_fp8);
            # e is indexed g' = s*392 + c, so this write is contiguous.
            e_fp8 = cp.tile([128, 2, 400], FP8)
            rowsum = cp.tile([128, 1], F32)
            nc.scalar.activation(
                e_fp8[:, :, 0:NCH],
                accs[(RD - 1) % 2][:].rearrange("p (s c) -> p s c", s=2),
                AF.Exp, accum_out=rowsum[:])

            S_all = cp.tile([128, 1], F32)
            nc.gpsimd.partition_all_reduce(
                S_all[:], rowsum[:], 128, bass.bass_isa.ReduceOp.add)
            invS = cp.tile([128, 1], F32)
            nc.vector.reciprocal(invS[:], S_all[:])

            # ---- gate preact: 5 gates x (8 h-chunks + x) --------------
            pre_ps = pp.tile([128, 5], F32, tag="pre")
            for g in range(5):
                for kc in range(8):
                    nc.tensor.matmul(pre_ps[:, g:g + 1],
                                     wg_sb[:, g, kc, :],
                                     h8_ap[:, kc:kc + 1],
                                     start=(kc == 0), stop=False)
                nc.tensor.matmul(pre_ps[:, g:g + 1],
                                 wx_sb[:, g, :], x_ap,
                                 start=False, stop=True)

            pre_sb = cp.tile([128, 5], F32)
            nc.vector.tensor_add(pre_sb[:], pre_ps[:], b_ap)
            th = cp.tile([128, 4], F32)
            nc.scalar.activation(th[:], pre_sb[:, 0:4], AF.Tanh, scale=0.5)
            gates = cp.tile([128, 4], F32)
            nc.vector.tensor_scalar(gates[:], th[:], 0.5, 0.5, OP.mult, OP.add)
            cnew = cp.tile([128, 1], F32)
            nc.scalar.activation(cnew[:], pre_sb[:, 4:5], AF.Tanh)

            # ---- big matvec: p_slice = e @ vals (fp8 DoubleRow) -------
            p_ps = pp.tile([1, 128], F32, tag="p")
            for b in range(NBLK):
                v = vblk[b]
                for c in range(CPB):
                    gchunk = b * CPB + c
                    lhsT = e_fp8[:, :, gchunk:gchunk + 1]
                    nc.tensor.matmul(p_ps[:], lhsT, v[:, c, :, :],
                                     start=(gchunk == 0),
                                     stop=(gchunk == NCH - 1),
                                     perf_mode=DR)

            # ---- transpose p to [128,1], finish the cell --------------
            p_row = cp.tile([1, 128], F32)
            nc.vector.tensor_copy(p_row[:], p_ps[:])
            p_t = pp.tile([128, 1], F32, tag="tr")
            nc.tensor.transpose(p_t[:], p_row[:], ones_sb[:])

            m_t = cp.tile([128, 1], F32)
            nc.scalar.activation(m_t[:], p_t[:], AF.Tanh, scale=invS[:, 0:1])

            out_sb = cp.tile([128, 2], F32)
            t2 = cp.tile([128, 1], F32)
            nc.vector.tensor_mul(t2[:], gates[:, 1:2], cnew[:])
            ct0 = cp.tile([128, 1], F32)
            nc.vector.scalar_tensor_tensor(
                ct0[:], gates[:, 0:1], c2_ap, t2[:], OP.mult, OP.add)
            nc.vector.scalar_tensor_tensor(
                out_sb[:, 1:2], gates[:, 3:4], m_t[:, 0:1], ct0[:],
                OP.mult, OP.add)
            tct = cp.tile([128, 1], F32)
            nc.scalar.activation(tct[:], out_sb[:, 1:2], AF.Tanh)
            nc.vector.tensor_mul(out_sb[:, 0:1], gates[:, 2:3], tct[:])

            nc.sync.dma_start(out_hc[:], out_sb[:])

    nc.compile()
    return nc


def _get_nc():
    if "nc" not in _CACHE:
        _CACHE["nc"] = _build()
    return _CACHE["nc"]


def _prep_in_maps(x_t, h, c, keys, vals, W_i2h, b_i2h, W_h2h, b_h2h,
                  W_ih, b_ih, W_actor, b_actor, W_critic, b_critic, pick_arm):
    f = np.float32
    x_t = np.asarray(x_t, f)
    h = np.asarray(h, f).reshape(-1)          # [H]
    c = np.asarray(c, f).reshape(-1)          # [H]
    keys = np.asarray(keys, f)
    vals = np.asarray(vals, f)

    pa = int(np.asarray(pick_arm))
    start = min(max(pa * RD, 0), IN_DIM - RD)  # jax dynamic_slice clamping
    q = x_t[0, start:start + RD]
    qn = np.linalg.norm(q)
    qhat = (q / max(qn, 1e-30)).astype(f)

    # khat_i = k_i / max(||k_i||, 1e-8/||q||) reproduces the reference's
    # denom = max(||k||*||q||, 1e-8); pad rows get -40*qhat so their
    # similarity is -40 and the softmax weight underflows to exactly 0.
    kn = np.linalg.norm(keys, axis=1, keepdims=True)
    khat = keys / np.maximum(kn, 1e-8 / max(qn, 1e-30))
    khat_pad = np.empty((PAD_D, RD), f)
    khat_pad[:D] = khat
    khat_pad[D:] = -40.0 * qhat
    # column order g' = s*392 + c  <->  row block 128*(2c+s)
    gp = np.arange(G10)
    g10_of_gp = 2 * (gp % NCH) + gp // NCH
    # [128, RD, G10]: partition p, coordinate r, column g'
    khat_t = np.ascontiguousarray(
        khat_pad.reshape(G10, 128, RD)[g10_of_gp].transpose(1, 2, 0)
        .reshape(128, RD * G10)).astype(FP8_NP)

    vals_f8 = vals.astype(FP8_NP)
    b5 = (np.asarray(b_i2h, f) + np.asarray(b_h2h, f))
    W_h2h = np.asarray(W_h2h, f)
    W_i2h = np.asarray(W_i2h, f)
    BF = ml_dtypes.bfloat16
    pk16 = np.zeros((128, 9), BF)
    pk16[:, 0:8] = h.reshape(8, 128).T
    pk16[0:IN_DIM, 8] = x_t[0]

    in_maps = []
    for k in range(N_CORES):
        cols = slice(128 * k, 128 * (k + 1))
        vp = np.zeros((PAD_D, 128), FP8_NP)
        vp[:D] = vals_f8[:, cols]
        # [128, NCH, 2, 128]: partition p, chunk c, sub s -> row 256c+128s+p
        vals_s = np.ascontiguousarray(
            vp.reshape(NCH, 2, 128, 128).transpose(2, 0, 1, 3)
            .reshape(128, NCH * 2 * 128))

        rows = (np.arange(5)[:, None] * H + 128 * k
                + np.arange(128)[None, :]).reshape(-1)   # [5*128]
        Wslice = W_h2h[rows].reshape(5, 128, 8, 128)     # [g, p, kc, kk]
        wg = np.ascontiguousarray(
            Wslice.transpose(3, 0, 2, 1).reshape(128, 5 * 8 * 128)).astype(BF)
        Wxs = W_i2h[rows].reshape(5, 128, IN_DIM)        # [g, p, r]
        wx = np.ascontiguousarray(
            Wxs.transpose(2, 0, 1).reshape(IN_DIM, 5 * 128)).astype(BF)
        pk32 = np.empty((128, 16), f)
        pk32[:, 0:RD] = qhat
        pk32[:, RD:RD + 5] = b5[rows].reshape(5, 128).T
        pk32[:, RD + 5] = c[cols]

        in_maps.append({
            "vals_s": vals_s,
            "khat": khat_t,
            "pk32": pk32,
            "pk16": pk16,
            "wg": wg,
            "wx": wx,
        })
    return in_maps


def _postprocess(results, W_ih, b_ih, W_actor, b_actor, W_critic, b_critic):
    f = np.float32
    h_t = np.concatenate([r["out_hc"][:, 0] for r in results]).astype(f)
    c_t = np.concatenate([r["out_hc"][:, 1] for r in results]).astype(f)
    hh = np.maximum(np.asarray(W_ih, f) @ h_t + np.asarray(b_ih, f), 0.0)
    logits = np.asarray(W_actor, f) @ hh + np.asarray(b_actor, f)
    v = np.asarray(W_critic, f) @ hh + np.asarray(b_critic, f)
    m = logits.max()
    ex = np.exp(logits - m)
    pi = (ex / ex.sum()).astype(f)
    a = int(np.argmax(np.log(pi) + GUMBEL))
    logp = np.float32(np.log(pi[a]))
    return np.concatenate([pi, v, [logp], h_t, c_t]).astype(f)


def kernel(**inputs) -> np.ndarray:
    nc = _get_nc()
    in_maps = _prep_in_maps(**inputs)
    res = run_bass_kernel_spmd(
        nc, in_maps, core_ids=list(range(N_CORES)),
        **_CACHE.get("run_kwargs", {}))
    _CACHE["last_results"] = res
    return _postprocess(res.results,
                        inputs["W_ih"], inputs["b_ih"],
                        inputs["W_actor"], inputs["b_actor"],
                        inputs["W_critic"], inputs["b_critic"])


# revision 9
# speedup vs baseline: 1.9763x; 1.9763x over previous
"""Trainium2 Bass kernel for CompositionalTwoArmedAgent (DND-LSTM A2C step).

Strategy (8 NeuronCores, column-sharded DND, ZERO device collectives):
  - vals is sharded COLUMN-wise: core k owns all 100k rows x hidden cols
    [128k:128(k+1)], stored fp8e4 in a row-chunked DoubleRow layout. Each
    core computes its own slice of p = e @ vals completely locally.
  - keys are host-normalized (khat = k/||k||, fp8) and replicated; each
    core computes the full similarity vector e = exp(khat @ qhat) with a
    10-step DVE multiply-accumulate chain + fused exp/rowsum on ScalarE.
    Pad rows store khat = -40*qhat so their weight underflows to exactly 0.
  - Cosine sims are bounded in [-1,1] => no softmax max-pass needed.
  - The LSTM gates are sliced to match: core k computes preact rows
    {g*1024 + 128k + p} for the 5 gates via 45 small stationary-weight
    matmuls (contraction over the replicated h / x), so f,i,o,r,c_new,
    c_t, h_t for its slice need no cross-core data at all.
  - The big matvec p_slice = e @ vals_slice runs as 392 DoubleRow fp8
    matmuls (256 rows per step, 2 MACs/PE/cycle) accumulating in PSUM.
  - All loads ride ONE HWDGE ring (sync) in need-order so completion
    semaphores fire as soon as each tensor lands (two rings round-robin
    at packet granularity and starve each other's completions).
  - e is indexed g' = s*392 + c (s = DoubleRow subtile, c = chunk) so the
    ScalarE exp writes it contiguously while the per-chunk ldweights AP
    keeps the required 16-byte-multiple subtile stride.
  - Host gathers the 8 [h_t|c_t] slices and applies the tiny A2C head
    (relu GEMV + 2-class softmax + fixed-key categorical sample), as the
    baseline already did for the softmax/sample.
"""

import ml_dtypes
import numpy as np

import concourse.bacc as bacc
import concourse.bass as bass
import concourse.mybir as mybir
import concourse.tile as tile
from concourse.bass_utils import run_bass_kernel_spmd

N_CORES = 8
D, RD, H, IN_DIM, A = 100000, 10, 1024, 14, 2
PAD_D = 100352          # 392 * 256
NCH = 392               # DoubleRow chunks (256 rows each)
G10 = 784               # 128-row groups (= 2 * NCH)
NBLK = 7                # vals DMA blocks
CPB = NCH // NBLK       # chunks per block
F32 = mybir.dt.float32
BF16 = mybir.dt.bfloat16
FP8 = mybir.dt.float8e4
FP8_NP = ml_dtypes.float8_e4m3

# jax.random.gumbel(jax.random.key(1), (2,), float32) — fixed constants of the
# reference's categorical sample (verified against jax.random.categorical).
GUMBEL = np.array([0.5325072, -0.01641824], np.float32)

_CACHE = {}


def _input_specs():
    return [
        ("vals_s", [128, NCH * 2 * 128], FP8),   # DoubleRow-chunked vals shard
        ("khat", [128, RD * G10], FP8),          # normalized keys, r-major
        ("pk32", [128, 16], F32),                # q_bc(10) | b_sl(5) | c2_sl(1)
        ("pk16", [128, 9], BF16),                # h_col8(8) | x_col(col 8)
        ("wg", [128, 5 * 8 * 128], BF16),        # gate h2h weights [g][kc][p]
        ("wx", [14, 5 * 128], BF16),             # gate i2h weights [g][p]
    ]


def _build():
    nc = bacc.Bacc("TRN2", target_bir_lowering=False, debug=False,
                   num_devices=N_CORES)
    d = {name: nc.dram_tensor(name, shp, dt, kind="ExternalInput")
         for name, shp, dt in _input_specs()}
    out_hc = nc.dram_tensor("out_hc", [128, 2], F32, kind="ExternalOutput")

    AF = mybir.ActivationFunctionType
    OP = mybir.AluOpType
    DR = mybir.MatmulPerfMode.DoubleRow

    with tile.TileContext(nc) as tc:
        with (
            tc.tile_pool(name="const", bufs=1) as cp,
            tc.tile_pool(name="ps", bufs=1, space="PSUM") as pp,
        ):
            # ---- single sync-ring DMA queue, in need-order ------------
            khat_sb = cp.tile([128, RD, G10], FP8)
            nc.sync.dma_start(
                khat_sb[:], d["khat"][:].rearrange("p (r g) -> p r g", g=G10))
            pk32_sb = cp.tile([128, 16], F32)
            nc.sync.dma_start(pk32_sb[:], d["pk32"][:])
            pk16_sb = cp.tile([128, 9], BF16)
            nc.sync.dma_start(pk16_sb[:], d["pk16"][:])
            wg_sb = cp.tile([128, 5, 8, 128], BF16)
            nc.sync.dma_start(
                wg_sb[:], d["wg"][:].rearrange("p (g c j) -> p g c j",
                                               c=8, j=128))
            wx_sb = cp.tile([14, 5, 128], BF16)
            nc.sync.dma_start(
                wx_sb[:], d["wx"][:].rearrange("p (g j) -> p g j", j=128))
            vblk = []
            for b in range(NBLK):
                # distinct name per block => distinct pool slot; a shared
                # name would alias one slot and serialize DMA behind compute
                v = cp.tile([128, CPB, 2, 128], FP8, name=f"vblk{b}")
                src = d["vals_s"][:, b * CPB * 256:(b + 1) * CPB * 256]
                nc.sync.dma_start(
                    v[:], src.rearrange("p (c s j) -> p c s j", s=2, j=128))
                vblk.append(v)

            q_ap = pk32_sb[:, 0:RD]
            b_ap = pk32_sb[:, RD:RD + 5]
            c2_ap = pk32_sb[:, RD + 5:RD + 6]
            h8_ap = pk16_sb[:, 0:8]
            x_ap = pk16_sb[0:14, 8:9]

            ones_sb = cp.tile([1, 1], F32)
            nc.vector.memset(ones_sb[:], 1.0)

            # ---- ACT table warmup (exp + tanh) ------------------------
            warm = cp.tile([1, 1], F32)
            nc.vector.memset(warm[:], 0.0)
            w2 = cp.tile([1, 1], F32)
            nc.scalar.activation(w2[:], warm[:], AF.Exp)
            nc.scalar.activation(w2[:], warm[:], AF.Tanh)

            # ---- e = exp(khat @ qhat), fused rowsum -------------------
            acc0 = cp.tile([128, G10], F32)
            acc1 = cp.tile([128, G10], F32)
            nc.vector.tensor_scalar(acc0[:], khat_sb[:, 0, :],
                                    q_ap[:, 0:1], None, OP.mult)
            accs = [acc0, acc1]
            for r in range(1, RD):
                nc.vector.scalar_tensor_tensor(
                    accs[r % 2][:], khat_sb[:, r, :], q_ap[:, r:r + 1],
                    accs[(r - 1) % 2][:], OP.mult, OP.add)
            # [128, 2, 400]: k-subtile-major so the DoubleRow ldweights AP
            # has a pair stride of 400 (multiple of 16, per s3_lw_dual_fp8);
            # e is indexed g' = s*392 + c, so this write is contiguous.
            e_fp8 = cp.tile([128, 2, 400], FP8)
            rowsum = cp.tile([128, 1], F32)
            nc.scalar.activation(
                e_fp8[:, :, 0:NCH],
                accs[(RD - 1) % 2][:].rearrange("p (s c) -> p s c", s=2),
                AF.Exp, accum_out=rowsum[:])

            S_all = cp.tile([128, 1], F32)
            nc.gpsimd.partition_all_reduce(
                S_all[:], rowsum[:], 128, bass.bass_isa.ReduceOp.add)
            invS = cp.tile([128, 1], F32)
            nc.vector.reciprocal(invS[:], S_all[:])

            # ---- gate preact: 5 gates x (8 h-chunks + x) --------------
            pre_ps = pp.tile([128, 5], F32, tag="pre")
            for g in range(5):
                for kc in range(8):
                    nc.tensor.matmul(pre_ps[:, g:g + 1],
                                     wg_sb[:, g, kc, :],
                                     h8_ap[:, kc:kc + 1],
                                     start=(kc == 0), stop=False)
                nc.tensor.matmul(pre_ps[:, g:g + 1],
                                 wx_sb[:, g, :], x_ap,
                                 start=False, stop=True)

            pre_sb = cp.tile([128, 5], F32)
            nc.vector.tensor_add(pre_sb[:], pre_ps[:], b_ap)
            th = cp.tile([128, 4], F32)
            nc.scalar.activation(th[:], pre_sb[:, 0:4], AF.Tanh, scale=0.5)
            gates = cp.tile([128, 4], F32)
            nc.vector.tensor_scalar(gates[:], th[:], 0.5, 0.5, OP.mult, OP.add)
            cnew = cp.tile([128, 1], F32)
            nc.scalar.activation(cnew[:], pre_sb[:, 4:5], AF.Tanh)

            # ---- big matvec: p_slice = e @ vals (fp8 DoubleRow) -------
            p_ps = pp.tile([1, 128], F32, tag="p")
            for b in range(NBLK):
                v = vblk[b]
                for c in range(CPB):
                    gchunk = b * CPB + c
                    lhsT = e_fp8[:, :, gchunk:gchunk + 1]
                    nc.tensor.matmul(p_ps[:], lhsT, v[:, c, :, :],
                                     start=(gchunk == 0),
                                     stop=(gchunk == NCH - 1),
                                     perf_mode=DR)

            # ---- transpose p to [128,1], finish the cell --------------
            p_row = cp.tile([1, 128], F32)
            nc.vector.tensor_copy(p_row[:], p_ps[:])
            p_t = pp.tile([128, 1], F32, tag="tr")
            nc.tensor.transpose(p_t[:], p_row[:], ones_sb[:])

            m_t = cp.tile([128, 1], F32)
            nc.scalar.activation(m_t[:], p_t[:], AF.Tanh, scale=invS[:, 0:1])

            out_sb = cp.tile([128, 2], F32)
            t2 = cp.tile([128, 1], F32)
            nc.vector.tensor_mul(t2[:], gates[:, 1:2], cnew[:])
            ct0 = cp.tile([128, 1], F32)
            nc.vector.scalar_tensor_tensor(
                ct0[:], gates[:, 0:1], c2_ap, t2[:], OP.mult, OP.add)
            nc.vector.scalar_tensor_tensor(
                out_sb[:, 1:2], gates[:, 3:4], m_t[:, 0:1], ct0[:],
                OP.mult, OP.add)
            tct = cp.tile([128, 1], F32)
            nc.scalar.activation(tct[:], out_sb[:, 1:2], AF.Tanh)
            nc.vector.tensor_mul(out_sb[:, 0:1], gates[:, 2:3], tct[:])

            nc.sync.dma_start(out_hc[:], out_sb[:])

    nc.compile()
    return nc


def _get_nc():
    if "nc" not in _CACHE:
        _CACHE["nc"] = _build()
    return _CACHE["nc"]


def _prep_in_maps(x_t, h, c, keys, vals, W_i2h, b_i2h, W_h2h, b_h2h,
                  W_ih, b_ih, W_actor, b_actor, W_critic, b_critic, pick_arm):
    f = np.float32
    x_t = np.asarray(x_t, f)
    h = np.asarray(h, f).reshape(-1)          # [H]
    c = np.asarray(c, f).reshape(-1)          # [H]
    keys = np.asarray(keys, f)
    vals = np.asarray(vals, f)

    pa = int(np.asarray(pick_arm))
    start = min(max(pa * RD, 0), IN_DIM - RD)  # jax dynamic_slice clamping
    q = x_t[0, start:start + RD]
    qn = np.linalg.norm(q)
    qhat = (q / max(qn, 1e-30)).astype(f)

    # khat_i = k_i / max(||k_i||, 1e-8/||q||) reproduces the reference's
    # denom = max(||k||*||q||, 1e-8); pad rows get -40*qhat so their
    # similarity is -40 and the softmax weight underflows to exactly 0.
    kn = np.linalg.norm(keys, axis=1, keepdims=True)
    khat = keys / np.maximum(kn, 1e-8 / max(qn, 1e-30))
    khat_pad = np.empty((PAD_D, RD), f)
    khat_pad[:D] = khat
    khat_pad[D:] = -40.0 * qhat
    # column order g' = s*392 + c  <->  row block 128*(2c+s)
    gp = np.arange(G10)
    g10_of_gp = 2 * (gp % NCH) + gp // NCH
    # [128, RD, G10]: partition p, coordinate r, column g'
    khat_t = np.ascontiguousarray(
        khat_pad.reshape(G10, 128, RD)[g10_of_gp].transpose(1, 2, 0)
        .reshape(128, RD * G10)).astype(FP8_NP)

    vals_f8 = vals.astype(FP8_NP)
    b5 = (np.asarray(b_i2h, f) + np.asarray(b_h2h, f))
    W_h2h = np.asarray(W_h2h, f)
    W_i2h = np.asarray(W_i2h, f)
    BF = ml_dtypes.bfloat16
    pk16 = np.zeros((128, 9), BF)
    pk16[:, 0:8] = h.reshape(8, 128).T
    pk16[0:IN_DIM, 8] = x_t[0]

    in_maps = []
    for k in range(N_CORES):
        cols = slice(128 * k, 128 * (k + 1))
        vp = np.zeros((PAD_D, 128), FP8_NP)
        vp[:D] = vals_f8[:, cols]
        # [128, NCH, 2, 128]: partition p, chunk c, sub s -> row 256c+128s+p
        vals_s = np.ascontiguousarray(
            vp.reshape(NCH, 2, 128, 128).transpose(2, 0, 1, 3)
            .reshape(128, NCH * 2 * 128))

        rows = (np.arange(5)[:, None] * H + 128 * k
                + np.arange(128)[None, :]).reshape(-1)   # [5*128]
        Wslice = W_h2h[rows].reshape(5, 128, 8, 128)     # [g, p, kc, kk]
        wg = np.ascontiguousarray(
            Wslice.transpose(3, 0, 2, 1).reshape(128, 5 * 8 * 128)).astype(BF)
        Wxs = W_i2h[rows].reshape(5, 128, IN_DIM)        # [g, p, r]
        wx = np.ascontiguousarray(
            Wxs.transpose(2, 0, 1).reshape(IN_DIM, 5 * 128)).astype(BF)
        pk32 = np.empty((128, 16), f)
        pk32[:, 0:RD] = qhat
        pk32[:, RD:RD + 5] = b5[rows].reshape(5, 128).T
        pk32[:, RD + 5] = c[cols]

        in_maps.append({
            "vals_s": vals_s,
            "khat": khat_t,
            "pk32": pk32,
            "pk16": pk16,
            "wg": wg,
            "wx": wx,
        })
    return in_maps


def _postprocess(results, W_ih, b_ih, W_actor, b_actor, W_critic, b_critic):
    f = np.float32
    h_t = np.concatenate([r["out_hc"][:, 0] for r in results]).astype(f)
    c_t = np.concatenate([r["out_hc"][:, 1] for r in results]).astype(f)
    hh = np.maximum(np.asarray(W_ih, f) @ h_t + np.asarray(b_ih, f), 0.0)
    logits = np.asarray(W_actor, f) @ hh + np.asarray(b_actor, f)
    v = np.asarray(W_critic, f) @ hh + np.asarray(b_critic, f)
    m = logits.max()
    ex = np.exp(logits - m)
    pi = (ex / ex.sum()).astype(f)
    a = int(np.argmax(np.log(pi) + GUMBEL))
    logp = np.float32(np.log(pi[a]))
    return np.concatenate([pi, v, [logp], h_t, c_t]).astype(f)


def kernel(**inputs) -> np.ndarray:
    nc = _get_nc()
    in_maps = _prep_in_maps(**inputs)
    res = run_bass_kernel_spmd(
        nc, in_maps, core_ids=list(range(N_CORES)),
        **_CACHE.get("run_kwargs", {}))
    _CACHE["last_results"] = res
    return _postprocess(res.results,
                        inputs["W_ih"], inputs["b_ih"],
                        inputs["W_actor"], inputs["b_actor"],
                        inputs["W_critic"], inputs["b_critic"])
